# revision 31
# baseline (speedup 1.0000x reference)
"""Trainium2 Bass kernel for NeuralSumProductModel (LDPC sum-product decoder).

Contract: kernel(**inputs) takes FULL inputs (llr [512,8192] f32,
var_index [24576] i32, chk_index [24576] i32) and returns the FULL
output [5, 512, 8192] f32, matching reference.reference().

Design v3 (per NeuronCore, batch sharded 512 -> 8 x 64): batch-on-free
layout + dma_gather. ap_gather moves 4B per index (~27ns/idx); dma_gather
moves a 256B row (64 f32 = one batch row) per descriptor at ~0.34ns/desc
generation and DMA-bus execution, so all permutation traffic rides DMA.

  - SBUF check layout: partition p owns checks {ct*128+p : ct in [0,32)},
    cols [ct][e][b] (e in [0,6) edge slot, b in [0,64) batch).
  - msg_e = out_prev[var(e)] - ext_prev[e]: out rows live in DRAM OUTR
    [8192, 64]; dma_gather pulls row var(e) for each edge slot.
  - check phase: 4 pieces x 8 check-tiles (3072 cols), baseline numerics
    (tanh, sign, abs, ln, strided reduce6, phi involution, sign via
    reduce-mult). New ext written to SBUF EXTSB (positional reuse next
    iteration) and streamed to DRAM EXTR [24576, 64] rows r=p*192+ct*6+e.
  - var phase: dma_gather pulls ext rows at each var's 3 edge positions
    -> VG [p, vt, s, b]; vsum = reduce over s; out = vsum + x; written
    back to OUTR (v-major rows) and to out_d (batch-major) from a
    transposed copy.
"""

import os
import sys

import numpy as np

for _p in ("/opt/trn_rl_repo", "/root/.axon_site/_ro/trn_rl_repo"):
    if os.path.isdir(_p) and _p not in sys.path:
        sys.path.insert(0, _p)

N_VAR, N_CHK, DV, DC = 8192, 4096, 3, 6
E = N_VAR * DV  # 24576
BATCH, N_ITER, N_CORES = 512, 5, 8
BC = BATCH // N_CORES           # 64 batch rows per core
NCT = N_CHK // 128              # 32 check tiles
NVT = N_VAR // 128              # 64 var blocks per partition (v = p*64 + vt)
NP_CHK = 4                      # check pieces per iteration
CTP = NCT // NP_CHK             # 8 check tiles per piece
PW = CTP * DC * BC              # 3072 cols per piece
W = NCT * DC * BC               # 12288 cols total (ext per partition)

GCH = 768                       # dma_gather rows per call: 49 descs/engine
                                # fits the 64-desc ring (1024 = 65 descs does not)

EPS = 1e-12
_C = np.float32(1.0) - np.float32(1e-7)
TCLIP = float(np.float32((np.float32(1.0) - _C) / (np.float32(1.0) + _C)))

_CACHE = {}
_LAST_RESULTS = None


def _wrap(stream):
    """Pack an index stream [n] -> wrapped [128, n//16], replicated across
    the 8 gpsimd cores (dma_gather uses one shared stream)."""
    st = np.asarray(stream, np.int16)
    n = st.shape[0]
    assert n % 16 == 0
    core = st.reshape(n // 16, 16).T     # [16, n//16]
    return np.tile(core, (8, 1))


def _build_indices(vi, ci):
    """Host-side graph preprocessing. Returns dict of wrapped index planes."""
    order = np.argsort(ci, kind="stable")          # check-major edge list
    cm_var = vi[order].astype(np.int64)            # var of each cm edge
    pos_of_edge = np.empty(E, np.int64)
    pos_of_edge[order] = np.arange(E)
    edges_of_var = np.argsort(vi, kind="stable").reshape(N_VAR, DV)
    pos_var = pos_of_edge[edges_of_var]            # [N_VAR, 3] cm positions

    # msg gather: OUTR row = var id of edge slot (c = ct*128 + p, e);
    # output slot j = (ct_loc*6 + e)*128 + p within each piece.
    ixm = np.zeros(E, np.int64)
    for pc in range(NP_CHK):
        for jl in range(CTP * DC):
            ctl, e = jl // DC, jl % DC
            c = (pc * CTP + ctl) * 128 + np.arange(128)
            ixm[pc * CTP * DC * 128 + jl * 128:
                pc * CTP * DC * 128 + (jl + 1) * 128] = cm_var[c * DC + e]
    planes = {"ixm": _wrap(ixm)}

    # var gather: EXTR row of var v's s-th edge; v = p*64 + vt,
    # output slot j = (vt*3 + s)*128 + p.
    vidx = np.zeros(DV * N_VAR, np.int64)
    for vt in range(NVT):
        for s in range(DV):
            p = np.arange(128)
            v = p * NVT + vt
            j = pos_var[v, s]                      # cm position
            c, e = j // DC, j % DC
            r = (c % 128) * (NCT * DC) + (c // 128) * DC + e
            vidx[(vt * DV + s) * 128 + p] = r
    planes["vidx"] = _wrap(vidx)
    return planes


def _build_bass():
    import concourse.bass as bass
    import concourse.tile as tile
    from concourse import bacc, mybir
    from contextlib import ExitStack

    dt = mybir.dt
    F32, BF16, I16 = dt.float32, dt.bfloat16, dt.int16
    ALU = mybir.AluOpType
    ACT = mybir.ActivationFunctionType
    AX = mybir.AxisListType

    # 4 SWDGE queues: each dma_gather call's await_space blocks only on its
    # own ring, so desc-gen and DMA execution overlap across queues.
    nc = bacc.Bacc("TRN2", target_bir_lowering=False, debug=False,
                   num_swdge_queues=4)

    llr_d = nc.dram_tensor("llr", [BC, N_VAR], F32, kind="ExternalInput").ap()
    ixm_d = nc.dram_tensor("ixm", [128, E // 16], I16,
                           kind="ExternalInput").ap()
    vidx_d = nc.dram_tensor("vidx", [128, DV * N_VAR // 16], I16,
                            kind="ExternalInput").ap()
    out_d = nc.dram_tensor("out", [N_ITER, BC, N_VAR], F32,
                           kind="ExternalOutput").ap()
    outr = nc.dram_tensor("outr", [N_VAR, BC], F32, kind="Internal").ap()
    extr = nc.dram_tensor("extr", [E, BC], F32, kind="Internal").ap()

    outr_flat = outr[:, :].rearrange("(p n) k -> p (n k)", p=128)
    extr_flat = extr[:, :].rearrange("(p n) k -> p (n k)", p=128)
    llr_bv = llr_d[:, :].rearrange("b (p vt) -> p b vt", p=128)

    with tile.TileContext(nc) as tc, ExitStack() as ctx:
        big = ctx.enter_context(tc.tile_pool(name="big", bufs=1))

        arena = big.tile([128, W], F32, tag="arena")         # 48KB
        extsb = big.tile([128, W], F32, tag="extsb")         # 48KB
        sgt = big.tile([128, 2 * PW], BF16, tag="sgt")       # 12KB
        out_v = big.tile([128, N_VAR // 2], F32, tag="out_v")   # 16KB
        out_b = big.tile([128, N_VAR // 2], F32, tag="out_b")   # 16KB
        xsb = big.tile([128, N_VAR // 2], F32, tag="xsb")    # 16KB (b-major)
        xsb_v = big.tile([128, N_VAR // 2], F32, tag="xsb_v")  # x, v-major
        csum = big.tile([128, 2 * CTP * BC], F32, tag="csum")
        cpt = big.tile([128, 2 * CTP * BC], BF16, tag="cpt")
        scs = big.tile([128, 4 * CTP * BC], F32, tag="scs")
        epst = big.tile([128, 1], F32, tag="epst")
        ixm_t = big.tile([128, E // 16], I16, tag="ixm_t")
        vidx_t = big.tile([128, DV * N_VAR // 16], I16, tag="vidx_t")

        TA = [arena[:, 0:PW], arena[:, PW:2 * PW]]
        TB = [arena[:, 2 * PW:3 * PW], arena[:, 3 * PW:4 * PW]]
        SG = [sgt[:, 0:PW], sgt[:, PW:2 * PW]]
        CS = [csum[:, 0:CTP * BC], csum[:, CTP * BC:2 * CTP * BC]]
        CP = [cpt[:, 0:CTP * BC], cpt[:, CTP * BC:2 * CTP * BC]]

        nc.vector.memset(epst[:], EPS)
        nc.vector.memset(extsb[:], 0.0)
        nc.sync.dma_start(ixm_t[:], ixm_d[:])
        nc.sync.dma_start(vidx_t[:], vidx_d[:])
        nc.sync.dma_start(xsb[:].rearrange("p (b vt) -> p b vt", b=BC),
                          llr_bv)

        # x in v-major layout (one-time transpose copy), and OUTR := x rows
        xsb_vv = xsb[:].rearrange("p (b vt) -> p vt b", b=BC)
        xv_vv = xsb_v[:].rearrange("p (vt b) -> p vt b", vt=NVT)
        ovv = out_v[:].rearrange("p (vt b) -> p vt b", vt=NVT)
        nc.vector.tensor_scalar_add(xv_vv, xsb_vv, 0.0)
        nc.sync.dma_start(outr_flat, xsb_v[:])

        for it in range(N_ITER):
            for pc in range(NP_CHK):
                k = pc % 2
                cl = slice(pc * PW, (pc + 1) * PW)
                ta, tb, sg, cs, cp = TA[k], TB[k], SG[k], CS[k], CP[k]

                nip = CTP * DC * 128        # 6144 gathered rows per piece
                # SWDGE ring caps a single dma_gather at ~1024 descriptors
                # (larger calls wedge the device); chunk the gather.
                for g0 in range(0, nip, GCH):
                    dsl = ta[:, (g0 // 128) * BC:((g0 + GCH) // 128) * BC]
                    nc.gpsimd.dma_gather(
                        dsl.rearrange("p (n k) -> p n k", k=BC), outr[:, :],
                        ixm_t[:, (pc * nip + g0) // 16:
                              (pc * nip + g0 + GCH) // 16],
                        num_idxs=GCH, num_idxs_reg=GCH, elem_size=BC,
                        queue_num=(g0 // GCH) % 4)

                # msg = gather(out_prev) - ext_prev
                nc.vector.tensor_tensor(tb, ta, extsb[:, cl],
                                        op=ALU.subtract)
                nc.scalar.activation(ta, tb, ACT.Tanh, scale=0.5)
                nc.scalar.activation(sg, ta, ACT.Sign)
                nc.scalar.activation(tb, ta, ACT.Abs)
                nc.scalar.activation(ta, tb, ACT.Ln, bias=epst[:])

                # per-edge-slot slices are [p, ct, b] with contiguous b runs;
                # slice-wise ops avoid the ~3.5x DVE strided-reduce penalty.
                la_e = [ta.rearrange("p (ct e b) -> p ct e b", ct=CTP, e=DC)
                        [:, :, e, :] for e in range(DC)]
                sg_e = [sg.rearrange("p (ct e b) -> p ct e b", ct=CTP, e=DC)
                        [:, :, e, :] for e in range(DC)]
                cs6 = cs.rearrange("p (ct b) -> p ct b", ct=CTP)
                cp6 = cp.rearrange("p (ct b) -> p ct b", ct=CTP)
                SM = CTP * BC
                tA, tB, tC, tD = (scs[:, i * SM:(i + 1) * SM].rearrange(
                    "p (ct b) -> p ct b", ct=CTP) for i in range(4))
                # csum tree: ((e0+e1)+(e2+e3)) + (e4+e5), no aliasing
                nc.vector.tensor_tensor(tA, la_e[0], la_e[1], op=ALU.add)
                nc.vector.tensor_tensor(tB, la_e[2], la_e[3], op=ALU.add)
                nc.vector.tensor_tensor(tC, la_e[4], la_e[5], op=ALU.add)
                nc.vector.tensor_tensor(tD, tA, tB, op=ALU.add)
                nc.vector.tensor_tensor(cs6, tD, tC, op=ALU.add)
                # sign product tree
                nc.vector.tensor_tensor(tA, sg_e[0], sg_e[1], op=ALU.mult)
                nc.vector.tensor_tensor(tB, sg_e[2], sg_e[3], op=ALU.mult)
                nc.vector.tensor_tensor(tC, sg_e[4], sg_e[5], op=ALU.mult)
                nc.vector.tensor_tensor(tD, tA, tB, op=ALU.mult)
                nc.vector.tensor_tensor(cp6, tD, tC, op=ALU.mult)

                dd_e = [tb.rearrange("p (ct e b) -> p ct e b", ct=CTP, e=DC)
                        [:, :, e, :] for e in range(DC)]
                for e in range(DC):
                    nc.vector.tensor_tensor(dd_e[e], cs6, la_e[e],
                                            op=ALU.subtract)

                nc.scalar.activation(ta, tb, ACT.Tanh, scale=-0.5)
                nc.vector.tensor_scalar_max(tb, ta, TCLIP)
                nc.scalar.activation(ta, tb, ACT.Ln)

                se_e = dd_e
                for e in range(DC):
                    nc.vector.tensor_tensor(se_e[e], sg_e[e], cp6,
                                            op=ALU.mult)
                nc.vector.scalar_tensor_tensor(
                    extsb[:, cl], ta, -1.0, tb, op0=ALU.mult, op1=ALU.mult)

                nc.sync.dma_start(extr_flat[:, cl], extsb[:, cl])

            # ---- var phase ----
            for g0 in range(0, DV * N_VAR, GCH):
                dsl = arena[:, (g0 // 128) * BC:((g0 + GCH) // 128) * BC]
                nc.gpsimd.dma_gather(
                    dsl.rearrange("p (n k) -> p n k", k=BC), extr[:, :],
                    vidx_t[:, g0 // 16:(g0 + GCH) // 16],
                    num_idxs=GCH, num_idxs_reg=GCH, elem_size=BC,
                    queue_num=(g0 // GCH) % 4)
            # out = x + s0 + s1 + s2 via contiguous-output slice adds;
            # final V-layout lands in out_b's storage, B-layout in out_v's.
            vg_s = [arena[:].rearrange("p (vt s b) -> p vt s b",
                                       vt=NVT, s=DV)[:, :, s, :]
                    for s in range(DV)]
            obv = out_b[:].rearrange("p (vt b) -> p vt b", vt=NVT)
            nc.vector.tensor_tensor(obv, vg_s[0], xv_vv, op=ALU.add)
            nc.vector.tensor_tensor(ovv, obv, vg_s[1], op=ALU.add)
            nc.vector.tensor_tensor(obv, ovv, vg_s[2], op=ALU.add)
            if it + 1 < N_ITER:
                nc.sync.dma_start(outr_flat, out_b[:])
            # transpose copy to batch-major and write the iteration output
            ovb = out_v[:].rearrange("p (b vt) -> p vt b", b=BC)
            nc.vector.tensor_scalar_add(ovb, obv, 0.0)
            od_bv = out_d[it].rearrange("b (p vt) -> p b vt", p=128)
            nc.sync.dma_start(od_bv,
                              out_v[:].rearrange("p (b vt) -> p b vt", b=BC))

    nc.compile()
    return nc


def _numpy_fallback(llr, vi, ci):
    x = llr.T.astype(np.float32)
    scattered = x[vi]
    ext = np.zeros_like(scattered)
    outs = []
    for _ in range(N_ITER):
        vsum = np.zeros((N_VAR, x.shape[1]), np.float32)
        np.add.at(vsum, vi, ext)
        msg = (vsum[vi] - ext) + scattered
        t = np.tanh(msg * 0.5)
        la = np.log(np.abs(t) + EPS)
        sg = np.sign(t)
        cs = np.zeros((N_CHK, x.shape[1]), np.float32)
        np.add.at(cs, ci, la)
        cpr = np.ones((N_CHK, x.shape[1]), np.float32)
        np.multiply.at(cpr, ci, sg)
        loo = np.exp(cs[ci] - la) * (cpr[ci] * sg)
        loo = np.clip(loo, -float(_C), float(_C))
        ext = 2.0 * np.arctanh(loo)
        vs2 = np.zeros((N_VAR, x.shape[1]), np.float32)
        np.add.at(vs2, vi, ext)
        outs.append((vs2 + x).T)
    return np.stack(outs)


def kernel(llr, var_index, chk_index):
    llr = np.asarray(llr, np.float32)
    vi = np.asarray(var_index, np.int64).ravel()
    ci = np.asarray(chk_index, np.int64).ravel()
    assert llr.shape == (BATCH, N_VAR) and vi.shape == (E,) and ci.shape == (E,)

    regular = (np.array_equal(np.bincount(vi, minlength=N_VAR),
                              np.full(N_VAR, DV))
               and np.array_equal(np.bincount(ci, minlength=N_CHK),
                                  np.full(N_CHK, DC)))
    if not regular:
        return _numpy_fallback(llr, vi, ci).astype(np.float32)

    key = ("k3", hash(vi.tobytes()), hash(ci.tobytes()))
    if key not in _CACHE:
        planes = _build_indices(vi, ci)
        nc = _build_bass()
        _CACHE[key] = (nc, planes)
    nc, planes = _CACHE[key]

    from concourse.bass_utils import run_bass_kernel_spmd
    in_maps = []
    for c in range(N_CORES):
        m = {nm: np.ascontiguousarray(v) for nm, v in planes.items()}
        m["llr"] = np.ascontiguousarray(llr[c * BC:(c + 1) * BC, :])
        in_maps.append(m)
    trace = os.environ.get("BASS_KERNEL_TRACE", "0") == "1"
    res = run_bass_kernel_spmd(nc, in_maps, list(range(N_CORES)), trace=trace)
    global _LAST_RESULTS
    _LAST_RESULTS = res
    out = np.concatenate([res.results[c]["out"] for c in range(N_CORES)],
                         axis=1)
    return np.ascontiguousarray(out, dtype=np.float32)


if __name__ == "__main__":
    sys.path.insert(0, os.path.dirname(os.path.abspath(__file__)))
    import reference
    inputs = {k: np.asarray(v) for k, v in reference.setup_inputs().items()}
    exp = np.asarray(reference.reference(**inputs))
    got = kernel(**inputs)
    err = np.max(np.abs(got - exp)) / (np.max(np.abs(exp)) + 1e-30)
    print("Relative error:", err)
